# revision 1
# baseline (speedup 1.0000x reference)
"""Trainium2 Bass kernel for nn_HGraphAttentionLayer (GAT-style layer, 8 NeuronCores).

Math (reference):
  feats[h,n,o]  = concat(input[:5000] @ proj_rna[h], input[5000:] @ proj_dis[h])
  s_src[h,n]    = feats[h,n,:] @ score_src[h];  s_tgt likewise
  attn[h,i,j]   = softmax_over_i( mask[i,j] + leaky_relu(s_src[h,i]+s_tgt[h,j], 0.2) )
  vals[i,o]     = mean_h( sum_j attn[h,i,j] * feats[h,j,o] )
  out           = elu( instancenorm(vals) + input @ residual_w.T )

Sharding: each of the 8 cores owns N/8 = 1024 query rows (i). The softmax
reduces over i (axis 1), so each core computes partial column sums d[h,j]
over its rows; an AllGather per j-stripe completes d. The bmm contracts
over j with e held in [j_partitions, i_free] layout (mask loaded transposed
via the DMA xbar after an fp32->bf16 pre-pass). vals^T accumulates in PSUM
across all stripes. InstanceNorm stats use one tiny AllReduce.
"""
import numpy as np

N, F, H, O = 8192, 256, 4, 128
N_CORES = 8
MY_N = N // N_CORES          # 1024 rows per core
N_RNA = 5000
SLOPE = 0.2
EPS = 1e-5
N_STRIPES = 8
SJ = N // N_STRIPES          # 1024 j per stripe
JT = SJ // 128               # 8 j-tiles per stripe
NCH = N // 128               # 64 n-chunks
FC = F // 128                # 2 f-chunks
SPLIT_CH = N_RNA // 128      # chunk 39 contains the rna/dis boundary
SPLIT_ROW = N_RNA - SPLIT_CH * 128  # row 8 within chunk 39

_cached = {}


def _build():
    import concourse.bass as bass
    import concourse.bacc as bacc
    import concourse.mybir as mybir
    import concourse.tile as tile

    f32 = mybir.dt.float32
    bf16 = mybir.dt.bfloat16
    Alu = mybir.AluOpType
    Act = mybir.ActivationFunctionType

    nc = bacc.Bacc("TRN2", target_bir_lowering=False, debug=False,
                   enable_asserts=False, num_devices=N_CORES)

    # ---- I/O -----------------------------------------------------------
    mask_rows = nc.dram_tensor("mask_rows", [MY_N, N], f32, kind="ExternalInput").ap()
    in_rna = nc.dram_tensor("in_rna", [MY_N, F], f32, kind="ExternalInput").ap()
    in_dis = nc.dram_tensor("in_dis", [MY_N, F], f32, kind="ExternalInput").ap()
    input_full = nc.dram_tensor("input_full", [N, F], f32, kind="ExternalInput").ap()
    proj_rna = nc.dram_tensor("proj_rna", [H, F, O], f32, kind="ExternalInput").ap()
    proj_dis = nc.dram_tensor("proj_dis", [H, F, O], f32, kind="ExternalInput").ap()
    score_src = nc.dram_tensor("score_src", [H, O, 1], f32, kind="ExternalInput").ap()
    score_tgt = nc.dram_tensor("score_tgt", [H, O, 1], f32, kind="ExternalInput").ap()
    residual_w = nc.dram_tensor("residual_w", [O, F], f32, kind="ExternalInput").ap()
    identf_in = nc.dram_tensor("identf", [128, 128], f32, kind="ExternalInput").ap()
    sel39_in = nc.dram_tensor("sel39", [128, 1], f32, kind="ExternalInput").ap()
    invsel39_in = nc.dram_tensor("invsel39", [128, 1], f32, kind="ExternalInput").ap()
    out_dram = nc.dram_tensor("out", [O, MY_N], f32, kind="ExternalOutput").ap()

    RG = [list(range(N_CORES))]

    with tile.TileContext(nc) as tc:
        with (
            tc.tile_pool(name="const", bufs=1) as constp,
            tc.tile_pool(name="pro", bufs=3) as pro,
            tc.tile_pool(name="feats_sb", bufs=6) as featsp,
            tc.tile_pool(name="dpool", bufs=3) as dpool,
            tc.tile_pool(name="ps_work", bufs=2, space="PSUM") as ps_work,
            tc.tile_pool(name="ps_s", bufs=2, space="PSUM") as ps_s,
            tc.tile_pool(name="ps_vals", bufs=1, space="PSUM") as ps_vals,
            tc.tile_pool(name="dram", bufs=1, space="DRAM") as dram,
        ):
            # ---- DRAM scratch ------------------------------------------
            maskb = [dram.tile([MY_N, SJ], bf16, tag=f"maskb{s}", name=f"maskb{s}")
                     for s in range(N_STRIPES)]
            inputT_dram = dram.tile([FC, 128, N], bf16, tag="inTd", name="inTd")
            feats_dram = dram.tile([NCH, 128, H * 128], bf16, tag="featsd", name="featsd")
            d_in = [dram.tile([128, 32], f32, tag=f"din{s}", name=f"din{s}")
                    for s in range(N_STRIPES)]
            d_out = [dram.tile([128 * N_CORES, 32], f32, tag=f"dout{s}", name=f"dout{s}")
                     for s in range(N_STRIPES)]
            st_in = dram.tile([1, 32], f32, tag="stin", name="stin")
            st_out = dram.tile([1, 32], f32, tag="stout", name="stout")
            dum_in = dram.tile([1, 16], f32, tag="dumin", name="dumin")
            dum_out = dram.tile([1, 16], f32, tag="dumout", name="dumout")
            arow_dram = dram.tile([H, MY_N], f32, tag="arowd", name="arowd")

            # ---- constants ---------------------------------------------
            identf = constp.tile([128, 128], f32, tag="identf", name="identf")
            nc.sync.dma_start(identf[:], identf_in)
            identb = constp.tile([128, 128], bf16, tag="identb", name="identb")
            nc.vector.tensor_copy(identb[:], identf[:])
            ones_col = constp.tile([128, 1], f32, tag="ones_col", name="ones_col")
            nc.vector.memset(ones_col[:], 1.0)
            ones_row = constp.tile([1, 512], f32, tag="ones_row", name="ones_row")
            nc.vector.memset(ones_row[:], 1.0)
            sel39 = constp.tile([128, 1], f32, tag="sel39", name="sel39")
            nc.sync.dma_start(sel39[:], sel39_in)
            invsel39 = constp.tile([128, 1], f32, tag="invsel39", name="invsel39")
            nc.sync.dma_start(invsel39[:], invsel39_in)

            # warm up the collective stack early (one-time comm init ~70us
            # overlaps the prologue instead of stalling stripe 0)
            zr = constp.tile([1, 16], f32, tag="zr", name="zr")
            nc.vector.memset(zr[:], 0.0)
            nc.sync.dma_start(dum_in[:], zr[:])
            nc.gpsimd.collective_compute(
                "AllReduce", Alu.add, replica_groups=RG,
                ins=[dum_in.opt()], outs=[dum_out.opt()])

            # ---- per-core row shards (rna/dis zero-masked) --------------
            rnaT = [constp.tile([128, MY_N], bf16, tag=f"rnaT{fc}", name=f"rnaT{fc}")
                    for fc in range(FC)]
            disT = [constp.tile([128, MY_N], bf16, tag=f"disT{fc}", name=f"disT{fc}")
                    for fc in range(FC)]
            rowsT = [constp.tile([128, MY_N], bf16, tag=f"rowsT{fc}", name=f"rowsT{fc}")
                     for fc in range(FC)]
            for src_ap, dstT in ((in_rna, rnaT), (in_dis, disT)):
                for ic in range(MY_N // 128):
                    raw = pro.tile([128, F], f32, tag="raw_in", name="raw_in")
                    nc.sync.dma_start(raw[:], src_ap[ic * 128:(ic + 1) * 128, :])
                    rawb = pro.tile([128, F], bf16, tag="rawb_in", name="rawb_in")
                    nc.vector.tensor_copy(rawb[:], raw[:])
                    for fc in range(FC):
                        tp = ps_work.tile([128, 128], bf16, tag="tp", name="tp")
                        nc.tensor.transpose(tp[:], rawb[:, fc * 128:(fc + 1) * 128], identb[:])
                        nc.vector.tensor_copy(dstT[fc][:, ic * 128:(ic + 1) * 128], tp[:])
            for fc in range(FC):
                nc.vector.tensor_add(rowsT[fc][:], rnaT[fc][:], disT[fc][:])

            # ---- projections -> bf16 -----------------------------------
            projb = {}
            for tname, pap in (("rna", proj_rna), ("dis", proj_dis)):
                for h in range(H):
                    for fc in range(FC):
                        praw = pro.tile([128, O], f32, tag="praw", name="praw", bufs=2)
                        nc.sync.dma_start(praw[:], pap[h, fc * 128:(fc + 1) * 128, :])
                        pb = constp.tile([128, O], bf16, tag=f"pb_{tname}{h}{fc}",
                                         name=f"pb_{tname}{h}{fc}")
                        nc.vector.tensor_copy(pb[:], praw[:])
                        projb[(tname, h, fc)] = pb

            # ---- score vectors -> q[type][fc] = [128f, 8] bf16 ----------
            # cols 0..3 = src head h, 4..7 = tgt head h
            q_rhs = {(t, fc): constp.tile([128, 8], bf16, tag=f"q{t}{fc}", name=f"q{t}{fc}")
                     for t in ("rna", "dis") for fc in range(FC)}
            for si, sap in ((0, score_src), (1, score_tgt)):
                for h in range(H):
                    scol = pro.tile([128, 1], f32, tag="scol", name="scol", bufs=2)
                    nc.sync.dma_start(scol[:], sap[h])
                    tpq = ps_work.tile([128, 128], f32, tag="tp", name="tpq")
                    nc.tensor.transpose(tpq[0:1, :], scol[:], identf[:])
                    wrow = pro.tile([1, 128], f32, tag="wrow", name="wrow", bufs=2)
                    nc.vector.tensor_copy(wrow[:], tpq[0:1, :])
                    wb = pro.tile([128, 128], f32, tag="wb", name="wb", bufs=2)
                    nc.gpsimd.partition_broadcast(wb[:], wrow[:])
                    for tname in ("rna", "dis"):
                        for fc in range(FC):
                            qcol = pro.tile([128, 1], f32, tag="qcol", name="qcol", bufs=2)
                            qscr = pro.tile([128, O], f32, tag="qscr", name="qscr", bufs=2)
                            nc.vector.scalar_tensor_tensor(
                                qscr[:], projb[(tname, h, fc)][:], 1.0, wb[:],
                                op0=Alu.mult, op1=Alu.mult, accum_out=qcol[:])
                            nc.vector.tensor_copy(
                                q_rhs[(tname, fc)][:, si * 4 + h:si * 4 + h + 1], qcol[:])

            # ---- input transpose + s (all n); inputT spilled to DRAM ----
            # s_all[ch] = [128, 8] f32 (cols: src h0..3, tgt h0..3)
            s_all = [constp.tile([128, 8], f32, tag=f"sall{ch}", name=f"sall{ch}")
                     for ch in range(NCH)]

            def chunk_parts(ch):
                if ch < SPLIT_CH:
                    return [("rna", 0, 128)]
                if ch > SPLIT_CH:
                    return [("dis", 0, 128)]
                return [("rna", 0, SPLIT_ROW), ("dis", SPLIT_ROW, 128)]

            for ch in range(NCH):
                raw = pro.tile([128, F], f32, tag="raw_in", name="raw_in")
                nc.sync.dma_start(raw[:], input_full[ch * 128:(ch + 1) * 128, :])
                rawb = pro.tile([128, F], bf16, tag="rawb_in", name="rawb_in")
                nc.vector.tensor_copy(rawb[:], raw[:])
                int_ch = []
                for fc in range(FC):
                    tp = ps_work.tile([128, 128], bf16, tag="tp", name="tp")
                    nc.tensor.transpose(tp[:], rawb[:, fc * 128:(fc + 1) * 128], identb[:])
                    itc = pro.tile([128, 128], bf16, tag="int_ch", name="int_ch", bufs=4)
                    nc.vector.tensor_copy(itc[:], tp[:])
                    nc.sync.dma_start(inputT_dram[fc, :, ch * 128:(ch + 1) * 128], itc[:])
                    int_ch.append(itc)
                parts = chunk_parts(ch)
                tmpd = {}
                for tname, r0, r1 in parts:
                    ps_sc = ps_s.tile([128, 8], f32, tag="small", name="pssc")
                    for fc in range(FC):
                        nc.tensor.matmul(ps_sc[:], int_ch[fc][:], q_rhs[(tname, fc)][:],
                                         start=(fc == 0), stop=(fc == FC - 1))
                    if r0 == 0 and r1 == 128:
                        nc.vector.tensor_copy(s_all[ch][:], ps_sc[:])
                    else:
                        tmps = featsp.tile([128, 8], f32, tag="tmps", name="tmps", bufs=3)
                        nc.vector.tensor_copy(tmps[:], ps_sc[:])
                        tmpd[tname] = tmps
                if ch == SPLIT_CH:
                    t1s = featsp.tile([128, 8], f32, tag="blends", name="blends", bufs=2)
                    nc.vector.tensor_scalar_mul(t1s[:], tmpd["dis"][:], invsel39[:])
                    nc.vector.scalar_tensor_tensor(
                        s_all[ch][:], tmpd["rna"][:], sel39[:], t1s[:],
                        op0=Alu.mult, op1=Alu.add)

            # ---- s_src for my rows -> A_bcast[h] = [128, MY_N] bf16 -----
            for ic in range(MY_N // 128):
                ps_sr = ps_s.tile([128, 8], f32, tag="small", name="pssr")
                k = 0
                for tname, Tt in (("rna", rnaT), ("dis", disT)):
                    for fc in range(FC):
                        nc.tensor.matmul(ps_sr[:], Tt[fc][:, ic * 128:(ic + 1) * 128],
                                         q_rhs[(tname, fc)][:],
                                         start=(k == 0), stop=(k == 3))
                        k += 1
                srow = pro.tile([128, 8], f32, tag="srow", name="srow", bufs=2)
                nc.vector.tensor_copy(srow[:], ps_sr[:])
                tps = ps_work.tile([128, 128], f32, tag="tp", name="tps")
                nc.tensor.transpose(tps[0:8, :], srow[:], identf[:])
                srT = pro.tile([8, 128], f32, tag="srT", name="srT", bufs=2)
                nc.vector.tensor_copy(srT[:], tps[0:8, :])
                for h in range(H):
                    nc.sync.dma_start(arow_dram[h, ic * 128:(ic + 1) * 128], srT[h:h + 1, :])
            A_bcast = []
            for h in range(H):
                af = pro.tile([128, MY_N], f32, tag="af", name="af", bufs=2)
                nc.sync.dma_start(af[:], arow_dram[h:h + 1, :].partition_broadcast(128))
                ab = constp.tile([128, MY_N], bf16, tag=f"ab{h}", name=f"ab{h}")
                nc.vector.tensor_copy(ab[:], af[:])
                A_bcast.append(ab)

            # ---- residual weight transpose -----------------------------
            wrT = []
            wraw = pro.tile([128, F], f32, tag="wraw", name="wraw", bufs=1)
            nc.sync.dma_start(wraw[:], residual_w)
            wrawb = pro.tile([128, F], bf16, tag="wrawb", name="wrawb", bufs=1)
            nc.vector.tensor_copy(wrawb[:], wraw[:])
            for fc in range(FC):
                tpw = ps_work.tile([128, 128], bf16, tag="tp", name="tpw")
                nc.tensor.transpose(tpw[:], wrawb[:, fc * 128:(fc + 1) * 128], identb[:])
                wt = constp.tile([128, 128], bf16, tag=f"wrT{fc}", name=f"wrT{fc}")
                nc.vector.tensor_copy(wt[:], tpw[:])
                wrT.append(wt)

            # ---- full feats (4 heads batched per chunk, PE work) --------
            for ch in range(NCH):
                inTs = []
                for fc in range(FC):
                    itl = pro.tile([128, 128], bf16, tag="inT_ld", name="inT_ld", bufs=4)
                    nc.sync.dma_start(itl[:], inputT_dram[fc, :, ch * 128:(ch + 1) * 128])
                    inTs.append(itl)
                fsb_all = featsp.tile([128, H * 128], bf16, tag="fsb", name="fsb")
                parts = chunk_parts(ch)
                tmpd = {}
                for tname, r0, r1 in parts:
                    whole = (r0 == 0 and r1 == 128)
                    tf_list = []
                    for h in range(H):
                        ps_f = ps_work.tile([128, O], f32, tag="psf", name="psf")
                        for fc in range(FC):
                            nc.tensor.matmul(ps_f[:], inTs[fc][:], projb[(tname, h, fc)][:],
                                             start=(fc == 0), stop=(fc == FC - 1))
                        if whole:
                            nc.vector.tensor_copy(fsb_all[:, h * 128:(h + 1) * 128], ps_f[:])
                        else:
                            tmpf = featsp.tile([128, O], bf16, tag="tmpf", name="tmpf", bufs=9)
                            nc.vector.tensor_copy(tmpf[:], ps_f[:])
                            tf_list.append(tmpf)
                    if not whole:
                        tmpd[tname] = tf_list
                if ch == SPLIT_CH:
                    # row-wise blend: rows < SPLIT_ROW take rna, rest take dis
                    for h in range(H):
                        t1 = featsp.tile([128, O], bf16, tag="blend", name="blend", bufs=2)
                        nc.vector.tensor_scalar_mul(t1[:], tmpd["dis"][h][:], invsel39[:])
                        nc.vector.scalar_tensor_tensor(
                            fsb_all[:, h * 128:(h + 1) * 128], tmpd["rna"][h][:], sel39[:],
                            t1[:], op0=Alu.mult, op1=Alu.add)
                nc.sync.dma_start(feats_dram[ch], fsb_all[:])

            # ---- main loop over j-stripes ------------------------------
            stripep = tc.alloc_tile_pool(name="stripe", bufs=3)
            epool = tc.alloc_tile_pool(name="epool", bufs=3)
            gpool = tc.alloc_tile_pool(name="gpool", bufs=4)
            vals_ps = ps_vals.tile([128, MY_N], f32, tag="big", name="vals")

            for s in range(N_STRIPES):
                # pre-pass: fp32 mask rows -> bf16 scratch (this stripe's cols)
                for it in range(MY_N // 128):
                    nat = stripep.tile([128, SJ], f32, tag="nat", name="nat")
                    nc.sync.dma_start(nat[:], mask_rows[it * 128:(it + 1) * 128,
                                                        s * SJ:(s + 1) * SJ])
                    natb = stripep.tile([128, SJ], bf16, tag="natb", name="natb")
                    nc.vector.tensor_copy(natb[:], nat[:])
                    nc.sync.dma_start(maskb[s][it * 128:(it + 1) * 128, :], natb[:])

                d_all = dpool.tile([128, 32], f32, tag="dall", name="dall")
                e_tiles = {}
                for jt in range(JT):
                    mT = stripep.tile([128, MY_N], bf16, tag="mT", name="mT", bufs=4)
                    nc.sync.dma_start_transpose(mT[:], maskb[s][:, jt * 128:(jt + 1) * 128])
                    for h in range(H):
                        ch = s * JT + jt
                        z = epool.tile([128, MY_N], bf16, tag="z", name="z")
                        nc.vector.scalar_tensor_tensor(
                            z[:], mT[:], s_all[ch][:, 4 + h:5 + h], A_bcast[h][:],
                            op0=Alu.add, op1=Alu.add)
                        y = epool.tile([128, MY_N], bf16, tag="y", name="y")
                        if (jt * H + h) % 32 < 7:
                            nc.vector.scalar_tensor_tensor(
                                y[:], z[:], SLOPE, z[:], op0=Alu.mult, op1=Alu.max)
                        else:
                            nc.scalar.activation(y[:], z[:], Act.Prelu, alpha=SLOPE)
                        e = epool.tile([128, MY_N], bf16, tag="e", name="e", bufs=42)
                        nc.scalar.activation(e[:], y[:], Act.Exp,
                                             accum_out=d_all[:, h * 8 + jt:h * 8 + jt + 1])
                        e_tiles[(h, jt)] = e

                # complete d across cores (partial sums over i-rows)
                nc.sync.dma_start(d_in[s][:], d_all[:])
                nc.gpsimd.collective_compute(
                    "AllGather", Alu.bypass, replica_groups=RG,
                    ins=[d_in[s].opt()], outs=[d_out[s].opt()])
                dg = dpool.tile([128, 256], f32, tag="dg", name="dg")
                for r in range(N_CORES):
                    nc.sync.dma_start(dg[:, r * 32:(r + 1) * 32],
                                      d_out[s][r * 128:(r + 1) * 128, :])
                d_sum = dpool.tile([128, 32], f32, tag="dsum", name="dsum")
                nc.vector.tensor_add(d_sum[:], dg[:, 0:32], dg[:, 32:64])
                for r in range(2, N_CORES):
                    nc.vector.tensor_add(d_sum[:], d_sum[:], dg[:, r * 32:(r + 1) * 32])
                dinv = dpool.tile([128, 32], f32, tag="dinv", name="dinv")
                nc.vector.reciprocal(dinv[:], d_sum[:])

                # g = feats / d ; vals^T += g^T-contract-e
                for jt in range(JT):
                    ch = s * JT + jt
                    fst4 = gpool.tile([128, H * 128], bf16, tag="fst4", name="fst4")
                    nc.sync.dma_start(fst4[:], feats_dram[ch])
                    g4 = gpool.tile([128, H * 128], bf16, tag="g4", name="g4")
                    for h in range(H):
                        nc.vector.tensor_scalar_mul(
                            g4[:, h * 128:(h + 1) * 128], fst4[:, h * 128:(h + 1) * 128],
                            dinv[:, h * 8 + jt:h * 8 + jt + 1])
                    for h in range(H):
                        e = e_tiles[(h, jt)]
                        first = (s == 0) and h == 0 and jt == 0
                        last = (s == N_STRIPES - 1) and h == H - 1 and jt == JT - 1
                        nc.tensor.matmul(vals_ps[:, 0:512], g4[:, h * 128:(h + 1) * 128],
                                         e[:, 0:512], start=first, stop=last)
                        nc.tensor.matmul(vals_ps[:, 512:1024], g4[:, h * 128:(h + 1) * 128],
                                         e[:, 512:1024], start=first, stop=last)

            # ---- tail: instance norm + residual + elu ------------------
            gpool.release()
            epool.release()
            stripep.release()
            tailp = tc.alloc_tile_pool(name="tail", bufs=1)
            vs = tailp.tile([128, MY_N], f32, tag="vs", name="vs")
            srow1 = tailp.tile([128, 1], f32, tag="srow1", name="srow1")
            nc.scalar.activation(vs[:], vals_ps[:], Act.Copy, scale=0.25,
                                 accum_out=srow1[:])
            vsq = tailp.tile([128, MY_N], f32, tag="vsq", name="vsq")
            srow2 = tailp.tile([128, 1], f32, tag="srow2", name="srow2")
            nc.scalar.activation(vsq[:], vs[:], Act.Square, accum_out=srow2[:])

            ps1 = ps_s.tile([1, 1], f32, tag="small", name="ps1")
            nc.tensor.matmul(ps1[:], srow1[:], ones_col[:])
            ps2 = ps_s.tile([1, 1], f32, tag="small", name="ps2")
            nc.tensor.matmul(ps2[:], srow2[:], ones_col[:])
            stv = tailp.tile([1, 32], f32, tag="stv", name="stv")
            nc.vector.memset(stv[:], 0.0)
            nc.vector.tensor_copy(stv[0:1, 0:1], ps1[:])
            nc.vector.tensor_copy(stv[0:1, 16:17], ps2[:])
            nc.sync.dma_start(st_in[:], stv[:])
            nc.gpsimd.collective_compute(
                "AllReduce", Alu.add, replica_groups=RG,
                ins=[st_in.opt()], outs=[st_out.opt()])
            str_ = tailp.tile([1, 32], f32, tag="str", name="str")
            nc.sync.dma_start(str_[:], st_out[:])

            c = 1.0 / float(N * O)
            mu = tailp.tile([1, 1], f32, tag="mu", name="mu")
            nc.vector.tensor_scalar_mul(mu[:], str_[0:1, 0:1], c)
            m2 = tailp.tile([1, 1], f32, tag="m2", name="m2")
            nc.vector.tensor_scalar_mul(m2[:], str_[0:1, 16:17], c)
            mu2 = tailp.tile([1, 1], f32, tag="mu2", name="mu2")
            nc.vector.tensor_mul(mu2[:], mu[:], mu[:])
            var = tailp.tile([1, 1], f32, tag="var", name="var")
            nc.vector.tensor_sub(var[:], m2[:], mu2[:])
            vpe = tailp.tile([1, 1], f32, tag="vpe", name="vpe")
            nc.vector.tensor_scalar_add(vpe[:], var[:], EPS)
            sd = tailp.tile([1, 1], f32, tag="sd", name="sd")
            nc.scalar.activation(sd[:], vpe[:], Act.Sqrt)
            rstd = tailp.tile([1, 1], f32, tag="rstd", name="rstd")
            nc.vector.reciprocal(rstd[:], sd[:])
            negmurs = tailp.tile([1, 1], f32, tag="negmurs", name="negmurs")
            nc.vector.tensor_mul(negmurs[:], mu[:], rstd[:])
            nc.vector.tensor_scalar_mul(negmurs[:], negmurs[:], -1.0)

            a_col = tailp.tile([128, 1], f32, tag="acol", name="acol")
            nc.gpsimd.partition_broadcast(a_col[:], rstd[:])
            b_row = tailp.tile([1, 128], f32, tag="brow", name="brow")
            nc.scalar.activation(b_row[:], ones_row[0:1, 0:128], Act.Copy,
                                 scale=negmurs[:])

            r_ps = ps_vals.tile([128, MY_N], f32, tag="big", name="resid")
            for half in range(2):
                sl = slice(half * 512, (half + 1) * 512)
                for fc in range(FC):
                    nc.tensor.matmul(r_ps[:, sl], wrT[fc][:], rowsT[fc][:, sl],
                                     start=(fc == 0), stop=False)
                nc.tensor.matmul(r_ps[:, sl], b_row[:], ones_row[:],
                                 start=False, stop=True)

            pre = tailp.tile([128, MY_N], f32, tag="pre", name="pre")
            nc.vector.scalar_tensor_tensor(pre[:], vs[:], a_col[:], r_ps[:],
                                           op0=Alu.mult, op1=Alu.add)
            negp = tailp.tile([128, MY_N], f32, tag="negp", name="negp")
            nc.vector.tensor_scalar_min(negp[:], pre[:], 0.0)
            w = tailp.tile([128, MY_N], f32, tag="w", name="w")
            nc.scalar.activation(w[:], negp[:], Act.Exp)
            r1 = tailp.tile([128, MY_N], f32, tag="r1", name="r1")
            nc.vector.tensor_scalar_max(r1[:], pre[:], 0.0)
            outt = tailp.tile([128, MY_N], f32, tag="outt", name="outt")
            nc.vector.scalar_tensor_tensor(outt[:], w[:], -1.0, r1[:],
                                           op0=Alu.add, op1=Alu.add)
            nc.sync.dma_start(out_dram, outt[:])
            tailp.release()

    nc.compile()
    return nc


def _get_nc():
    if "nc" not in _cached:
        _cached["nc"] = _build()
    return _cached["nc"]


def kernel(input_mat, connectivity_mask, proj_rna, proj_dis, score_src,
           score_tgt, residual_w):
    from concourse.bass_utils import run_bass_kernel_spmd

    nc = _get_nc()
    input_mat = np.asarray(input_mat, np.float32)
    connectivity_mask = np.asarray(connectivity_mask, np.float32)
    ident = np.eye(128, dtype=np.float32)
    sel39 = (np.arange(128) < SPLIT_ROW).astype(np.float32)[:, None]
    rna_mask = (np.arange(N) < N_RNA).astype(np.float32)[:, None]
    in_rna_full = input_mat * rna_mask
    in_dis_full = input_mat * (1.0 - rna_mask)

    in_maps = []
    for k in range(N_CORES):
        r0, r1 = k * MY_N, (k + 1) * MY_N
        in_maps.append({
            "mask_rows": np.ascontiguousarray(connectivity_mask[r0:r1]),
            "in_rna": np.ascontiguousarray(in_rna_full[r0:r1]),
            "in_dis": np.ascontiguousarray(in_dis_full[r0:r1]),
            "input_full": input_mat,
            "proj_rna": np.asarray(proj_rna, np.float32),
            "proj_dis": np.asarray(proj_dis, np.float32),
            "score_src": np.asarray(score_src, np.float32),
            "score_tgt": np.asarray(score_tgt, np.float32),
            "residual_w": np.asarray(residual_w, np.float32),
            "identf": ident,
            "sel39": sel39,
            "invsel39": 1.0 - sel39,
        })

    res = run_bass_kernel_spmd(nc, in_maps, core_ids=list(range(N_CORES)))
    _cached["last_result"] = res
    out = np.empty((N, O), np.float32)
    for k in range(N_CORES):
        out[k * MY_N:(k + 1) * MY_N, :] = res.results[k]["out"].T
    return out



# revision 2
# speedup vs baseline: 1.0081x; 1.0081x over previous
"""Trainium2 Bass kernel v2 for nn_HGraphAttentionLayer (8 NeuronCores).

Reference math:
  feats[h,n,o]  = concat(input[:5000] @ proj_rna[h], input[5000:] @ proj_dis[h])
  s_src[h,n]    = feats[h,n,:] @ score_src[h];  s_tgt likewise
  e[h,j,i]      = exp(lrelu(s_src[h,i] + s_tgt[h,j] + M[i,j], 0.2))
                  (M additive {0,-1e9}; lrelu(-1e9) -> exp -> exact 0)
  d[h,j]        = sum_i e[h,j,i]   (softmax denominator, global over i)
  vals[i,o]     = mean_h( sum_j (feats[h,j,o]/d[h,j]) * e[h,j,i] )
  out           = elu( instancenorm(vals) + input @ residual_w.T )

Per-tile chain ([128 j, 1024 i] bf16), engine-balanced:
  zm = M'_jt + S_h                    DVE tensor_add            (~0.69us)
  y  = lrelu(zm + t_j)                ACT Prelu(bias=t_j)       (~1.13us)
       or on DVE: q1 = zm + t_j; q2 = 0.2*zm + 0.2*t_j; y = max(q1,q2)
  e  = Exp(y) with accum -> d         ACT                       (~1.41us)

Sharding: row sharding; core k owns target rows i in [k*1024,(k+1)*1024).
e is laid out [j_partitions, i_free]; the mask ships from host pre-transposed
(just a dtype/layout transform). d partials AllGather per half-stripe
(4 j-chunks x 4 heads), pipelined against the next half-stripe's elementwise.
feats are computed just-in-time per half-stripe; nothing spills to DRAM.
"""
import numpy as np

N, F, H, O = 8192, 256, 4, 128
N_CORES = 8
MY_N = N // N_CORES          # 1024 rows per core
N_RNA = 5000
SLOPE = 0.2
EPS = 1e-5
NCH = N // 128               # 64 j-chunks
FC = F // 128                # 2 f-chunks
SPLIT_CH = N_RNA // 128      # chunk 39 contains the rna/dis boundary
SPLIT_ROW = N_RNA - SPLIT_CH * 128  # row 8 within chunk 39
N_HS = 16                    # half-stripes
CPH = NCH // N_HS            # 4 chunks per half-stripe

_cached = {}


def _build():
    import concourse.bass as bass
    import concourse.bacc as bacc
    import concourse.mybir as mybir
    import concourse.tile as tile

    f32 = mybir.dt.float32
    bf16 = mybir.dt.bfloat16
    Alu = mybir.AluOpType
    Act = mybir.ActivationFunctionType

    nc = bacc.Bacc("TRN2", target_bir_lowering=False, debug=False,
                   enable_asserts=False, num_devices=N_CORES)

    # ---- I/O -----------------------------------------------------------
    maskT_in = nc.dram_tensor("maskT", [N, MY_N], bf16, kind="ExternalInput").ap()
    inputT_in = nc.dram_tensor("inputT", [FC, 128, N], bf16,
                               kind="ExternalInput").ap()
    # own-row inputT, zero-split by node type (for s_src of own rows)
    myrna_in = nc.dram_tensor("myrnaT", [FC, 128, MY_N], bf16,
                              kind="ExternalInput").ap()
    mydis_in = nc.dram_tensor("mydisT", [FC, 128, MY_N], bf16,
                              kind="ExternalInput").ap()
    proj_rna = nc.dram_tensor("proj_rna", [H, F, O], f32, kind="ExternalInput").ap()
    proj_dis = nc.dram_tensor("proj_dis", [H, F, O], f32, kind="ExternalInput").ap()
    score_src = nc.dram_tensor("score_src", [H, O, 1], f32, kind="ExternalInput").ap()
    score_tgt = nc.dram_tensor("score_tgt", [H, O, 1], f32, kind="ExternalInput").ap()
    residual_wT = nc.dram_tensor("residual_wT", [FC, 128, O], bf16,
                                 kind="ExternalInput").ap()
    identf_in = nc.dram_tensor("identf", [128, 128], f32, kind="ExternalInput").ap()
    sel39_in = nc.dram_tensor("sel39", [128, 1], f32, kind="ExternalInput").ap()
    invsel39_in = nc.dram_tensor("invsel39", [128, 1], f32, kind="ExternalInput").ap()
    out_dram = nc.dram_tensor("out", [O, MY_N], f32, kind="ExternalOutput").ap()

    RG = [list(range(N_CORES))]

    with tile.TileContext(nc) as tc:
        with (
            tc.tile_pool(name="const", bufs=1) as constp,
            tc.tile_pool(name="pro", bufs=3) as pro,
            tc.tile_pool(name="ps_work", bufs=1, space="PSUM") as ps_work,
            tc.tile_pool(name="ps_s", bufs=2, space="PSUM") as ps_s,
            tc.tile_pool(name="ps_feats", bufs=3, space="PSUM") as ps_feats,
            tc.tile_pool(name="ps_vals", bufs=1, space="PSUM") as ps_vals,
            tc.tile_pool(name="dram", bufs=1, space="DRAM") as dram,
        ):
            # ---- DRAM scratch ------------------------------------------
            d_in = [dram.tile([128, 16], f32, tag=f"din{s}", name=f"din{s}")
                    for s in range(N_HS)]
            d_out = [dram.tile([128 * N_CORES, 16], f32, tag=f"dout{s}",
                               name=f"dout{s}")
                     for s in range(N_HS)]
            arow_dram = dram.tile([H, MY_N], bf16, tag="arowd", name="arowd")
            st_in = dram.tile([1, 32], f32, tag="stin", name="stin")
            st_out = dram.tile([1, 32], f32, tag="stout", name="stout")
            dum_in = dram.tile([1, 16], f32, tag="dumin", name="dumin")
            dum_out = dram.tile([1, 16], f32, tag="dumout", name="dumout")

            # ---- constants ---------------------------------------------
            identf = constp.tile([128, 128], f32, tag="identf", name="identf")
            nc.sync.dma_start(identf[:], identf_in)
            ones_col = constp.tile([128, 1], f32, tag="ones_col", name="ones_col")
            nc.vector.memset(ones_col[:], 1.0)
            ones_row = constp.tile([1, 512], f32, tag="ones_row", name="ones_row")
            nc.vector.memset(ones_row[:], 1.0)
            sel39 = constp.tile([128, 1], f32, tag="sel39", name="sel39")
            nc.sync.dma_start(sel39[:], sel39_in)
            invsel39 = constp.tile([128, 1], f32, tag="invsel39", name="invsel39")
            nc.sync.dma_start(invsel39[:], invsel39_in)

            # warm up the collective stack early
            zr = constp.tile([1, 16], f32, tag="zr", name="zr")
            nc.vector.memset(zr[:], 0.0)
            nc.sync.dma_start(dum_in[:], zr[:])
            nc.gpsimd.collective_compute(
                "AllReduce", Alu.add, replica_groups=RG,
                ins=[dum_in.opt()], outs=[dum_out.opt()])

            # ---- projections -> bf16, 4 heads side by side -------------
            # projb_all[(t, fc)] = [128 f, 4h*128 o] so feats is one FD=512 mm
            projb_all = {}
            for tname, pap in (("rna", proj_rna), ("dis", proj_dis)):
                for fc in range(FC):
                    pb = constp.tile([128, H * O], bf16, tag=f"pb_{tname}{fc}",
                                     name=f"pb_{tname}{fc}")
                    for h in range(H):
                        praw = pro.tile([128, O], f32, tag="praw", name="praw",
                                        bufs=2)
                        nc.sync.dma_start(praw[:], pap[h, fc * 128:(fc + 1) * 128, :])
                        nc.vector.tensor_copy(pb[:, h * 128:(h + 1) * 128], praw[:])
                    projb_all[(tname, fc)] = pb

            # ---- residual weight (host pre-transposed) -----------------
            wrT = []
            for fc in range(FC):
                wt = constp.tile([128, O], bf16, tag=f"wrT{fc}", name=f"wrT{fc}")
                nc.sync.dma_start(wt[:], residual_wT[fc])
                wrT.append(wt)

            # ---- own-row inputT (zero-split) + residual rhs ------------
            myrnaT, mydisT, rowsT = [], [], []
            for fc in range(FC):
                ra = constp.tile([128, MY_N], bf16, tag=f"myrna{fc}",
                                 name=f"myrna{fc}")
                nc.sync.dma_start(ra[:], myrna_in[fc])
                myrnaT.append(ra)
                di = constp.tile([128, MY_N], bf16, tag=f"mydis{fc}",
                                 name=f"mydis{fc}")
                nc.sync.dma_start(di[:], mydis_in[fc])
                mydisT.append(di)
                rt = constp.tile([128, MY_N], bf16, tag=f"rowsT{fc}",
                                 name=f"rowsT{fc}")
                nc.vector.tensor_add(rt[:], ra[:], di[:])
                rowsT.append(rt)

            # ---- score vectors -> q[type][fc] = [128f, 8] bf16 ---------
            # q[f] = sum_o proj[h][f,o] * score[h][o], via PE with projT
            # cols 0..3 = src head h, 4..7 = tgt head h
            q_rhs = {(t, fc): constp.tile([128, 8], bf16, tag=f"q{t}{fc}",
                                          name=f"q{t}{fc}")
                     for t in ("rna", "dis") for fc in range(FC)}
            scols = {}
            for si, sap in ((0, score_src), (1, score_tgt)):
                for h in range(H):
                    sc = pro.tile([128, 1], f32, tag="scols", name="scols", bufs=8)
                    nc.sync.dma_start(sc[:], sap[h])
                    scols[(si, h)] = sc
            for tname, pap in (("rna", proj_rna), ("dis", proj_dis)):
                for fc in range(FC):
                    psq = ps_s.tile([128, 48], f32, tag="pss", name="psq")
                    for h in range(H):
                        praw2 = pro.tile([128, O], f32, tag="praw2", name="praw2",
                                         bufs=2)
                        nc.sync.dma_start(praw2[:],
                                          pap[h, fc * 128:(fc + 1) * 128, :])
                        tpp = ps_work.tile([128, 128], f32, tag="tp", name="tpp")
                        nc.tensor.transpose(tpp[:], praw2[:], identf[:])
                        pT = pro.tile([128, 128], f32, tag="pT", name="pT", bufs=2)
                        nc.vector.tensor_copy(pT[:], tpp[:])
                        for si in range(2):
                            nc.tensor.matmul(
                                psq[:, si * 4 + h:si * 4 + h + 1], pT[:],
                                scols[(si, h)][:], start=True, stop=True)
                    nc.vector.tensor_copy(q_rhs[(tname, fc)][:], psq[:, 0:8])

            # ---- own-row s_src -> S_h bcast tiles ----------------------
            for ic in range(MY_N // 128):
                ps_sr = ps_s.tile([128, 48], f32, tag="pss", name="pssr")
                k = 0
                for tname, Tt in (("rna", myrnaT), ("dis", mydisT)):
                    for fc in range(FC):
                        nc.tensor.matmul(ps_sr[:, 0:8],
                                         Tt[fc][:, ic * 128:(ic + 1) * 128],
                                         q_rhs[(tname, fc)][:],
                                         start=(k == 0), stop=(k == 3))
                        k += 1
                srow = pro.tile([128, 8], f32, tag="srow", name="srow", bufs=2)
                nc.vector.tensor_copy(srow[:], ps_sr[:, 0:8])
                tps = ps_work.tile([128, 128], f32, tag="tp", name="tps")
                nc.tensor.transpose(tps[0:8, :], srow[:], identf[:])
                srT = pro.tile([8, 128], bf16, tag="srT8", name="srT8", bufs=2)
                nc.vector.tensor_copy(srT[:], tps[0:8, :])
                for h in range(H):
                    nc.sync.dma_start(arow_dram[h, ic * 128:(ic + 1) * 128],
                                      srT[h:h + 1, :])
            # broadcast raw s_src rows (head h) to [128, MY_N] bf16
            S_b = []
            for h in range(H):
                sb = constp.tile([128, MY_N], bf16, tag=f"Sb{h}", name=f"Sb{h}")
                nc.sync.dma_start(sb[:],
                                  arow_dram[h:h + 1, :].partition_broadcast(128))
                S_b.append(sb)

            def chunk_type(ch):
                if ch < SPLIT_CH:
                    return "rna"
                if ch > SPLIT_CH:
                    return "dis"
                return "both"

            # ================= main loop over half-stripes ===============
            loop = tc.alloc_tile_pool(name="loop", bufs=3)
            epool = tc.alloc_tile_pool(name="epool", bufs=34)
            vals_ps = ps_vals.tile([128, MY_N], f32, tag="big", name="vals")
            first_mm = [True]
            pending = None

            for hs in range(N_HS):
                chunks = [hs * CPH + c for c in range(CPH)]
                # -- mask tile prefetch (issue before compute work) ------
                mTs = []
                for c, ch in enumerate(chunks):
                    mT = loop.tile([128, MY_N], bf16, tag="mT", name="mT", bufs=12)
                    nc.sync.dma_start(mT[:], maskT_in[ch * 128:(ch + 1) * 128, :])
                    mTs.append(mT)
                # -- inputT chunk loads + s + feats (JIT) ----------------
                pss = ps_s.tile([128, 48], f32, tag="pss", name=f"pss{hs}")
                fsb = []   # per chunk [128, H*128] bf16 feats
                for c, ch in enumerate(chunks):
                    its = []
                    for fc in range(FC):
                        itl = loop.tile([128, 128], bf16, tag="inT", name="inT",
                                        bufs=10)
                        nc.sync.dma_start(
                            itl[:], inputT_in[fc, :, ch * 128:(ch + 1) * 128])
                        its.append(itl)
                    ctype = chunk_type(ch)
                    # s-chunk: cols c*8..c*8+8 (and 40..48 for ch39's dis)
                    if ctype in ("rna", "dis"):
                        for fc in range(FC):
                            nc.tensor.matmul(pss[:, c * 8:(c + 1) * 8], its[fc][:],
                                             q_rhs[(ctype, fc)][:],
                                             start=(fc == 0), stop=(fc == FC - 1))
                    else:
                        for fc in range(FC):
                            nc.tensor.matmul(pss[:, c * 8:(c + 1) * 8], its[fc][:],
                                             q_rhs[("rna", fc)][:],
                                             start=(fc == 0), stop=(fc == FC - 1))
                        for fc in range(FC):
                            nc.tensor.matmul(pss[:, 40:48], its[fc][:],
                                             q_rhs[("dis", fc)][:],
                                             start=(fc == 0), stop=(fc == FC - 1))
                    # feats chunk: one FD=512 matmul per fc
                    psf = ps_feats.tile([128, 512], f32, tag="psf", name=f"psf{ch}")
                    if ctype in ("rna", "dis"):
                        for fc in range(FC):
                            nc.tensor.matmul(psf[:], its[fc][:],
                                             projb_all[(ctype, fc)][:],
                                             start=(fc == 0), stop=(fc == FC - 1))
                        fs = loop.tile([128, 512], bf16, tag="fsb", name="fsb",
                                       bufs=10)
                        nc.vector.tensor_copy(fs[:], psf[:])
                    else:
                        psf2 = ps_feats.tile([128, 512], f32, tag="psf",
                                             name=f"psf2{ch}")
                        for fc in range(FC):
                            nc.tensor.matmul(psf[:], its[fc][:],
                                             projb_all[("rna", fc)][:],
                                             start=(fc == 0), stop=(fc == FC - 1))
                            nc.tensor.matmul(psf2[:], its[fc][:],
                                             projb_all[("dis", fc)][:],
                                             start=(fc == 0), stop=(fc == FC - 1))
                        fs = loop.tile([128, 512], bf16, tag="fsb", name="fsb",
                                       bufs=10)
                        t1b = loop.tile([128, 512], bf16, tag="blendf",
                                        name="blendf", bufs=2)
                        nc.vector.tensor_scalar_mul(t1b[:], psf2[:], invsel39[:])
                        nc.vector.scalar_tensor_tensor(
                            fs[:], psf[:], sel39[:], t1b[:],
                            op0=Alu.mult, op1=Alu.add)
                    fsb.append(fs)

                # s columns to SBUF (+ ch39 blend), t5 = 0.2*t
                scol = loop.tile([128, 48], f32, tag="scol", name=f"scol{hs}",
                                 bufs=3)
                nc.vector.tensor_copy(scol[:], pss[:])
                if SPLIT_CH in chunks:
                    c39 = chunks.index(SPLIT_CH)
                    tb = loop.tile([128, 8], f32, tag="blends", name="blends",
                                   bufs=2)
                    nc.vector.tensor_scalar_mul(tb[:], scol[:, 40:48], invsel39[:])
                    tr = loop.tile([128, 8], f32, tag="blendr", name="blendr",
                                   bufs=2)
                    nc.vector.tensor_copy(tr[:], scol[:, c39 * 8:(c39 + 1) * 8])
                    nc.vector.scalar_tensor_tensor(
                        scol[:, c39 * 8:(c39 + 1) * 8],
                        tr[:], sel39[:], tb[:],
                        op0=Alu.mult, op1=Alu.add)
                t5 = loop.tile([128, 16], f32, tag="t5", name=f"t5{hs}", bufs=3)
                for c in range(CPH):
                    nc.vector.tensor_scalar_mul(
                        t5[:, c * 4:(c + 1) * 4],
                        scol[:, c * 8 + 4:c * 8 + 8], SLOPE)

                # -- elementwise: 16 tiles; prev half-stripe's bmm emitted
                #    between chunk 1 and chunk 2 so its AllReduce latency
                #    hides under this half-stripe's elementwise work.
                dcol = loop.tile([128, 16], f32, tag="dcol", name=f"dcol{hs}",
                                 bufs=3)
                e_tiles = {}

                def elementwise(c, ch):
                    mT = mTs[c]
                    for h in range(H):
                        tcol = scol[:, c * 8 + 4 + h:c * 8 + 4 + h + 1]
                        t5col = t5[:, c * 4 + h:c * 4 + h + 1]
                        zm = loop.tile([128, MY_N], bf16, tag="zm", name="zm",
                                       bufs=6)
                        nc.vector.tensor_add(zm[:], mT[:], S_b[h][:])
                        y = loop.tile([128, MY_N], bf16, tag="y", name="y", bufs=6)
                        if (c * H + h) % 16 < 8:
                            nc.scalar.activation(y[:], zm[:], Act.Prelu,
                                                 bias=tcol, scale=1.0, alpha=SLOPE)
                        else:
                            q1 = loop.tile([128, MY_N], bf16, tag="q1", name="q1",
                                           bufs=4)
                            nc.vector.tensor_scalar_add(q1[:], zm[:], tcol)
                            q2 = loop.tile([128, MY_N], bf16, tag="q2", name="q2",
                                           bufs=4)
                            nc.vector.tensor_scalar(q2[:], zm[:], SLOPE, t5col,
                                                    op0=Alu.mult, op1=Alu.add)
                            nc.vector.tensor_max(y[:], q1[:], q2[:])
                        e = epool.tile([128, MY_N], bf16, tag="e", name="e")
                        nc.scalar.activation(e[:], y[:], Act.Exp,
                                             accum_out=dcol[:, c * 4 + h:
                                                            c * 4 + h + 1])
                        e_tiles[(c, h)] = e

                def emit_bmm(st):
                    p_e, p_fsb, p_hs, p_last = st
                    # complete d for the previous half-stripe only now, so
                    # the AllGather latency hides under this half-stripe's
                    # elementwise and no DMA queue stalls on it early.
                    dg = loop.tile([128, 16, N_CORES], f32, tag="dg",
                                   name=f"dg{p_hs}", bufs=3)
                    for r in range(N_CORES):
                        nc.sync.dma_start(dg[:, :, r],
                                          d_out[p_hs][r * 128:(r + 1) * 128, :])
                    d_sum = loop.tile([128, 16], f32, tag="dsum",
                                      name=f"dsum{p_hs}", bufs=3)
                    nc.vector.tensor_reduce(d_sum[:], dg[:],
                                            mybir.AxisListType.X, Alu.add)
                    p_dinv = loop.tile([128, 16], f32, tag="dinv",
                                       name=f"dinv{p_hs}", bufs=3)
                    nc.vector.reciprocal(p_dinv[:], d_sum[:])
                    for c in range(CPH):
                        for h in range(H):
                            g4 = loop.tile([128, 128], bf16, tag="g4", name="g4",
                                           bufs=6)
                            nc.vector.tensor_scalar_mul(
                                g4[:], p_fsb[c][:, h * 128:(h + 1) * 128],
                                p_dinv[:, c * 4 + h:c * 4 + h + 1])
                            e = p_e[(c, h)]
                            last = p_last and c == CPH - 1 and h == H - 1
                            nc.tensor.matmul(vals_ps[:, 0:512], g4[:],
                                             e[:, 0:512],
                                             start=first_mm[0], stop=last)
                            nc.tensor.matmul(vals_ps[:, 512:1024], g4[:],
                                             e[:, 512:1024],
                                             start=first_mm[0], stop=last)
                            first_mm[0] = False

                for c, ch in enumerate(chunks[:3]):
                    elementwise(c, ch)
                if pending is not None:
                    emit_bmm(pending)
                    pending = None
                for c, ch in list(enumerate(chunks))[3:]:
                    elementwise(c, ch)

                # -- d AllGather (completed lazily in emit_bmm) ----------
                nc.sync.dma_start(d_in[hs][:], dcol[:])
                nc.gpsimd.collective_compute(
                    "AllGather", Alu.bypass, replica_groups=RG,
                    ins=[d_in[hs].opt()], outs=[d_out[hs].opt()])
                pending = (e_tiles, fsb, hs, hs == N_HS - 1)

            emit_bmm(pending)

            # ---- tail: instance norm + residual + elu ------------------
            epool.release()
            loop.release()
            tailp = tc.alloc_tile_pool(name="tail", bufs=1)
            vs = tailp.tile([128, MY_N], f32, tag="vs", name="vs")
            srow1 = tailp.tile([128, 1], f32, tag="srow1", name="srow1")
            nc.scalar.activation(vs[:], vals_ps[:], Act.Copy, scale=0.25,
                                 accum_out=srow1[:])
            vsq = tailp.tile([128, MY_N], f32, tag="vsq", name="vsq")
            srow2 = tailp.tile([128, 1], f32, tag="srow2", name="srow2")
            nc.scalar.activation(vsq[:], vs[:], Act.Square, accum_out=srow2[:])

            ps1 = ps_s.tile([128, 16], f32, tag="pss", name="ps1")
            nc.tensor.matmul(ps1[0:1, 0:1], srow1[:], ones_col[:])
            ps2 = ps_s.tile([128, 16], f32, tag="pss", name="ps2")
            nc.tensor.matmul(ps2[0:1, 0:1], srow2[:], ones_col[:])
            stv = tailp.tile([1, 32], f32, tag="stv", name="stv")
            nc.vector.memset(stv[:], 0.0)
            nc.vector.tensor_copy(stv[0:1, 0:1], ps1[0:1, 0:1])
            nc.vector.tensor_copy(stv[0:1, 16:17], ps2[0:1, 0:1])
            nc.sync.dma_start(st_in[:], stv[:])
            nc.gpsimd.collective_compute(
                "AllReduce", Alu.add, replica_groups=RG,
                ins=[st_in.opt()], outs=[st_out.opt()])
            str_ = tailp.tile([1, 32], f32, tag="str", name="str")
            nc.sync.dma_start(str_[:], st_out[:])

            c = 1.0 / float(N * O)
            mu = tailp.tile([1, 1], f32, tag="mu", name="mu")
            nc.vector.tensor_scalar_mul(mu[:], str_[0:1, 0:1], c)
            m2 = tailp.tile([1, 1], f32, tag="m2", name="m2")
            nc.vector.tensor_scalar_mul(m2[:], str_[0:1, 16:17], c)
            mu2 = tailp.tile([1, 1], f32, tag="mu2", name="mu2")
            nc.vector.tensor_mul(mu2[:], mu[:], mu[:])
            var = tailp.tile([1, 1], f32, tag="var", name="var")
            nc.vector.tensor_sub(var[:], m2[:], mu2[:])
            vpe = tailp.tile([1, 1], f32, tag="vpe", name="vpe")
            nc.vector.tensor_scalar_add(vpe[:], var[:], EPS)
            sd = tailp.tile([1, 1], f32, tag="sd", name="sd")
            nc.scalar.activation(sd[:], vpe[:], Act.Sqrt)
            rstd = tailp.tile([1, 1], f32, tag="rstd", name="rstd")
            nc.vector.reciprocal(rstd[:], sd[:])
            negmurs = tailp.tile([1, 1], f32, tag="negmurs", name="negmurs")
            nc.vector.tensor_mul(negmurs[:], mu[:], rstd[:])
            nc.vector.tensor_scalar_mul(negmurs[:], negmurs[:], -1.0)

            a_col = tailp.tile([128, 1], f32, tag="acol", name="acol")
            nc.gpsimd.partition_broadcast(a_col[:], rstd[:])
            b_row = tailp.tile([1, 128], f32, tag="brow", name="brow")
            nc.scalar.activation(b_row[:], ones_row[0:1, 0:128], Act.Copy,
                                 scale=negmurs[:])

            r_ps = ps_vals.tile([128, MY_N], f32, tag="big", name="resid")
            for half in range(2):
                sl = slice(half * 512, (half + 1) * 512)
                for fc in range(FC):
                    nc.tensor.matmul(r_ps[:, sl], wrT[fc][:], rowsT[fc][:, sl],
                                     start=(fc == 0), stop=False)
                nc.tensor.matmul(r_ps[:, sl], b_row[:], ones_row[:],
                                 start=False, stop=True)

            pre = tailp.tile([128, MY_N], f32, tag="pre", name="pre")
            nc.vector.scalar_tensor_tensor(pre[:], vs[:], a_col[:], r_ps[:],
                                           op0=Alu.mult, op1=Alu.add)
            negp = tailp.tile([128, MY_N], f32, tag="negp", name="negp")
            nc.vector.tensor_scalar_min(negp[:], pre[:], 0.0)
            w = tailp.tile([128, MY_N], f32, tag="w", name="w")
            nc.scalar.activation(w[:], negp[:], Act.Exp)
            r1 = tailp.tile([128, MY_N], f32, tag="r1", name="r1")
            nc.vector.tensor_scalar_max(r1[:], pre[:], 0.0)
            outt = tailp.tile([128, MY_N], f32, tag="outt", name="outt")
            nc.vector.scalar_tensor_tensor(outt[:], w[:], -1.0, r1[:],
                                           op0=Alu.add, op1=Alu.add)
            nc.sync.dma_start(out_dram, outt[:])
            tailp.release()

    nc.compile()
    return nc


def _get_nc():
    if "nc" not in _cached:
        _cached["nc"] = _build()
    return _cached["nc"]


def kernel(input_mat, connectivity_mask, proj_rna, proj_dis, score_src,
           score_tgt, residual_w):
    import ml_dtypes
    from concourse.bass_utils import run_bass_kernel_spmd

    bf = ml_dtypes.bfloat16
    nc = _get_nc()
    input_mat = np.asarray(input_mat, np.float32)
    connectivity_mask = np.asarray(connectivity_mask, np.float32)
    ident = np.eye(128, dtype=np.float32)
    sel39 = (np.arange(128) < SPLIT_ROW).astype(np.float32)[:, None]
    rna_mask = (np.arange(N) < N_RNA).astype(np.float32)[:, None]

    inputT = np.ascontiguousarray(input_mat.T).astype(bf)      # [F, N]
    in_rna_T = np.ascontiguousarray((input_mat * rna_mask).T).astype(bf)
    in_dis_T = np.ascontiguousarray((input_mat * (1.0 - rna_mask)).T).astype(bf)
    residual_wT_np = np.ascontiguousarray(
        np.asarray(residual_w, np.float32).T).astype(bf)       # [F, O]

    in_maps = []
    for k in range(N_CORES):
        r0, r1 = k * MY_N, (k + 1) * MY_N
        maskT_k = np.ascontiguousarray(
            connectivity_mask[r0:r1, :].T).astype(bf)          # [N, MY_N]
        in_maps.append({
            "maskT": maskT_k,
            "inputT": inputT.reshape(FC, 128, N),
            "myrnaT": np.ascontiguousarray(in_rna_T[:, r0:r1]).reshape(
                FC, 128, MY_N),
            "mydisT": np.ascontiguousarray(in_dis_T[:, r0:r1]).reshape(
                FC, 128, MY_N),
            "proj_rna": np.asarray(proj_rna, np.float32),
            "proj_dis": np.asarray(proj_dis, np.float32),
            "score_src": np.asarray(score_src, np.float32),
            "score_tgt": np.asarray(score_tgt, np.float32),
            "residual_wT": residual_wT_np.reshape(FC, 128, O),
            "identf": ident,
            "sel39": sel39,
            "invsel39": 1.0 - sel39,
        })

    res = run_bass_kernel_spmd(nc, in_maps, core_ids=list(range(N_CORES)))
    _cached["last_result"] = res
    out = np.empty((N, O), np.float32)
    for k in range(N_CORES):
        out[k * MY_N:(k + 1) * MY_N, :] = res.results[k]["out"].T
    return out
